# revision 1
# baseline (speedup 1.0000x reference)
"""CANLayer (cell attention) distributed Bass kernel for 8 TRN2 NeuronCores.

Strategy (graph/data parallel by destination cell, per sharding hint):
 - core k owns target nodes [k*LOCAL, (k+1)*LOCAL)
 - phase 1: per-core fused matmul x_k @ [W | W@A_s | W@A_d] for both edge sets
   + skip; builds 512B gather-table rows [xm_bf16(256B) | s_f32(16B) | pad]
 - 2x AllGather replicates the per-set tables to every core's HBM
 - edge phase per set: edges sorted by (src-chunk, target-window); big
   dma_gather calls (one DMA descriptor per edge, full HBM BW); per 128-edge
   tile build one-hot S [e,t] via DVE compare vs iota; S^T via PE transpose
   expands per-target t-values to edges; alpha -> lrelu -> exp on batched
   groups; scatter = matmul(lhsT=S, rhs=[M*e_att | e_att]) accumulated in
   PSUM per 128-target window; normalize by the denominator columns.
"""
import sys

if "/opt/trn_rl_repo" not in sys.path:
    sys.path.insert(0, "/opt/trn_rl_repo")

import numpy as np

TRACE = False          # test.py sets True to get exec_time_ns + perfetto
LAST_RESULT = {}       # test.py reads exec_time_ns etc. from here

NCORES = 8
WIN = 128              # targets per PSUM window
CALL_TILES = 8         # max 128-edge tiles per dma_gather call
RING = 4               # gather ring depth (pool bufs)
BINPACK = True         # permute targets into windows to balance edge counts
T_ZERO_DEBUG = False   # diagnostic (legacy flag, unused by route B)
COLL_BYPASS = False    # diagnostic: replace AllGather with local shard copy


def _binpack_windows(degs, local, nw):
    """degs: [n_kinds, local] per-target edge counts. Returns perm: array
    [local] -> window-major new position, such that each window's per-kind
    counts stay near/below budget. Greedy first-fit-decreasing."""
    nk = degs.shape[0]
    budgets = np.full((nw, nk), 500, np.int64)
    budgets[max(0, nw - 8):] = 6 * WIN
    counts = np.zeros((nw, nk), np.int64)
    nslots = np.zeros(nw, np.int64)
    win_of = np.full(local, -1, np.int64)
    order = np.argsort(-degs.sum(0), kind="stable")
    cap_slots = np.full(nw, WIN, np.int64)
    cap_slots[nw - 1] = local - (nw - 1) * WIN
    for t in order:
        d = degs[:, t]
        best, best_cost = -1, None
        for w in range(nw):
            if nslots[w] >= cap_slots[w]:
                continue
            over = np.maximum(counts[w] + d - budgets[w], 0).sum()
            slack = (budgets[w] - counts[w] - d).sum()
            cost = (over * 100000) - slack
            if best_cost is None or cost < best_cost:
                best, best_cost = w, cost
        counts[best] += d
        win_of[t] = best
        nslots[best] += 1
    # position within window
    perm = np.empty(local, np.int64)
    used = np.zeros(nw, np.int64)
    for t in range(local):
        w = win_of[t]
        perm[t] = w * WIN + used[w]
        used[w] += 1
    return perm


# ----------------------------------------------------------------------------
# host-side index preprocessing (pure layout/index manipulation)
# ----------------------------------------------------------------------------

def _preprocess(tgt, src, local, nw, sh, perms=None):
    """Shard one edge set by destination core; sort by (src-chunk, window).

    Static structure (caps, groups, calls) is shared across cores (maxed)
    as required for SPMD; per-core arrays carry indices + relative targets.
    perms[k] (optional) maps local target id -> permuted slot id.
    """
    per_core = []
    for k in range(NCORES):
        m = (tgt // local) == k
        s = src[m].astype(np.int64)
        tl = (tgt[m] - k * local).astype(np.int64)
        if perms is not None:
            tl = perms[k][tl]
        w = tl // WIN
        c = (s // local) // 4
        order = np.lexsort((tl, w, c))
        per_core.append((s[order], tl[order], w[order], c[order]))

    caps = np.zeros((2, nw), np.int64)
    for k in range(NCORES):
        s, tl, w, c = per_core[k]
        for cc in range(2):
            cnt = np.bincount(w[c == cc], minlength=nw)
            caps[cc] = np.maximum(caps[cc], (cnt + WIN - 1) // WIN)

    # groups in stream order; calls are CALL_TILES-sized slices of each
    # chunk run (groups may span calls)
    groups = []          # (cc, w, cap, tile_start)
    calls = []           # (cc, tile_start, n_tiles)
    t_idx = 0
    for cc in range(2):
        run_start = t_idx
        for w in range(nw):
            cap = int(caps[cc][w])
            if cap == 0:
                continue
            groups.append((cc, w, cap, t_idx))
            t_idx += cap
        for c0 in range(run_start, t_idx, CALL_TILES):
            calls.append((cc, c0, min(CALL_TILES, t_idx - c0)))
    T = t_idx

    cores = []
    for k in range(NCORES):
        s, tl, w, c = per_core[k]
        slots = T * 128
        src16 = np.zeros(slots, np.int16)          # pad -> idx 0 (valid row)
        trel = np.full(slots, -1.0, np.float32)    # pad -> -1 (no S match)
        for (cc, wg, cap, t0) in groups:
            sel = (c == cc) & (w == wg)
            n = int(sel.sum())
            off = t0 * 128
            ss = s[sel]
            if perms is not None:
                sloc = np.empty(len(ss), np.int64)
                for ks in range(NCORES):
                    mm = (ss // local) == ks
                    sloc[mm] = perms[ks][ss[mm] % local]
            else:
                sloc = ss % local
            src16[off:off + n] = (((ss // local) % 4) * sh + sloc).astype(np.int16)
            trel[off:off + n] = (tl[sel] - wg * WIN).astype(np.float32)
        tloc = np.zeros(slots, np.int32)
        for (cc, wg, cap, t0) in groups:
            blk = slice(t0 * 128, (t0 + cap) * 128)
            tloc[blk] = np.maximum(trel[blk].astype(np.int32), 0) + wg * WIN
        ii = np.arange(slots)
        idxarr = np.zeros((128, T * 8), np.int16)
        for g8 in range(8):
            idxarr[g8 * 16 + ii % 16, ii // 16] = src16
        trelarr = np.full((128, T), -1.0, np.float32)
        trelarr[ii % 128, ii // 128] = trel
        import ml_dtypes
        cores.append({"idx": idxarr, "trel": trelarr,
                      "trow": trel.reshape(1, T * 128).astype(ml_dtypes.bfloat16)})
    return caps, groups, calls, T, cores


def _block_diag_a(a):  # [H, C] -> [H*C, H] block diagonal (layout only)
    h, c = a.shape
    out = np.zeros((h * c, h), np.float32)
    for i in range(h):
        out[i * c:(i + 1) * c, i] = a[i]
    return out


# ----------------------------------------------------------------------------
# device kernel builder
# ----------------------------------------------------------------------------

def _build(meta):
    import concourse.bass as bass
    import concourse.bacc as bacc
    import concourse.mybir as mybir
    import concourse.tile as tile

    F32 = mybir.dt.float32
    BF16 = mybir.dt.bfloat16
    I16 = mybir.dt.int16
    I32 = mybir.dt.int32
    AL = mybir.AluOpType
    ACTF = mybir.ActivationFunctionType

    nw, sh = meta["nw"], meta["sh"]
    chunk_rows = 4 * sh
    eps_skip = meta["eps_skip"]

    nc = bacc.Bacc("TRN2", target_bir_lowering=False, debug=False,
                   num_devices=NCORES, num_swdge_queues=4)

    xT = nc.dram_tensor("xT", [128, sh], F32, kind="ExternalInput")
    Wcat = nc.dram_tensor("Wcat", [128, 384], F32, kind="ExternalInput")
    Acat = nc.dram_tensor("Acat", [128, 16], F32, kind="ExternalInput")
    out_ext = nc.dram_tensor("out", [sh, 128], F32, kind="ExternalOutput")

    sets = []
    for z, zn in enumerate("LU"):
        TZ = meta["T"][z]
        sets.append(dict(
            z=z, zn=zn,
            idx=nc.dram_tensor(f"idx{zn}", [128, TZ * 8], I16, kind="ExternalInput"),
            trel=nc.dram_tensor(f"trel{zn}", [128, TZ], F32, kind="ExternalInput"),
            trow=nc.dram_tensor(f"trow{zn}", [1, TZ * 128], BF16, kind="ExternalInput"),
            ag_in=nc.dram_tensor(f"agin{zn}", [sh, 128], F32),
            ag_out=nc.dram_tensor(f"agout{zn}", [NCORES * sh, 128], F32,
                                  addr_space="Shared"),
            caps=meta["caps"][z], groups=meta["groups"][z],
            calls=meta["calls"][z], T=TZ,
        ))

    rg = [list(range(NCORES))]

    with tile.TileContext(nc) as tc:
        with (
            tc.tile_pool(name="const", bufs=1) as constp,
            tc.tile_pool(name="p1", bufs=3) as p1,
            tc.tile_pool(name="gat", bufs=RING) as gatp,
            tc.tile_pool(name="work", bufs=6) as workp,
            tc.tile_pool(name="small", bufs=4) as smallp,
            tc.tile_pool(name="winb", bufs=1) as winp,
            tc.tile_pool(name="psA", bufs=2, space="PSUM") as psA,
            tc.tile_pool(name="psB", bufs=2, space="PSUM") as psB,
            tc.tile_pool(name="psC", bufs=2, space="PSUM") as psC,
            tc.tile_pool(name="psW", bufs=2, space="PSUM") as psW,
        ):
            # ---------------- constants ----------------
            wcat = constp.tile([128, 384], F32)
            nc.sync.dma_start(wcat[:], Wcat[:])
            wcat_bf = constp.tile([128, 384], BF16)
            nc.vector.tensor_copy(wcat_bf[:], wcat[:])
            acat = constp.tile([128, 16], F32)
            nc.sync.dma_start(acat[:], Acat[:])
            acat_bf = constp.tile([128, 16], BF16)
            nc.vector.tensor_copy(acat_bf[:], acat[:])

            iota_i = constp.tile([128, 128], I32)
            nc.gpsimd.iota(iota_i[:], [[1, 128]], base=0, channel_multiplier=0)
            iota_bf = constp.tile([128, 128], BF16)
            nc.vector.tensor_copy(iota_bf[:], iota_i[:])
            ones_row = constp.tile([1, 128], BF16)
            nc.vector.memset(ones_row[:], 1.0)
            iota_col = constp.tile([128, 1], F32)
            nc.gpsimd.iota(iota_col[:].bitcast(I32), [[1, 1]], base=0,
                           channel_multiplier=1)
            nc.vector.tensor_copy(iota_col[:], iota_col[:].bitcast(I32))
            iodiag = constp.tile([128, 128], I32)
            nc.gpsimd.iota(iodiag[:], [[1, 128]], base=0, channel_multiplier=-1)
            ident_bf = constp.tile([128, 128], BF16)
            nc.vector.tensor_single_scalar(ident_bf[:], iodiag[:], 0.0, AL.is_equal)

            for st in sets:
                zn = st["zn"]
                st["idx_sb"] = constp.tile([128, st["T"] * 8], I16, tag=f"idxsb{zn}", name=f"idxsb{zn}")
                nc.sync.dma_start(st["idx_sb"][:], st["idx"][:])
                st["trel_f"] = constp.tile([128, st["T"]], F32, tag=f"trelf{zn}", name=f"trelf{zn}")
                nc.sync.dma_start(st["trel_f"][:], st["trel"][:])


            # W_all = [Wl | Wl@As_l | Wl@Ad_l | Wu | ... | Wskip] bf16 [128,400]
            wall = constp.tile([128, 400], BF16)
            nc.vector.tensor_copy(wall[:, 0:128], wcat[:, 0:128])
            nc.vector.tensor_copy(wall[:, 136:264], wcat[:, 128:256])
            nc.vector.tensor_copy(wall[:, 272:400], wcat[:, 256:384])
            for z in range(2):
                pst = psC.tile([128, 128], BF16, tag="pbc")
                nc.tensor.transpose(pst[:], wcat_bf[:, z * 128:(z + 1) * 128],
                                    ident_bf[:])
                wTb = smallp.tile([128, 128], BF16, tag="wTb")
                nc.scalar.copy(wTb[:], pst[:])
                pwa = psW.tile([128, 132], F32, tag="pw")
                nc.tensor.matmul(pwa[:, 0:8], lhsT=wTb[:],
                                 rhs=acat_bf[:, z * 8:(z + 1) * 8],
                                 start=True, stop=True)
                nc.vector.tensor_copy(wall[:, 128 + z * 136:136 + z * 136],
                                      pwa[:, 0:8])

            # ---------------- persistent buffers ----------------
            out_acc = winp.tile([128, nw, 128], F32)
            tw = winp.tile([128, nw, 8], BF16)
            wacc = winp.tile([128, nw, 132], F32)

            # ---------------- phase 1 ----------------
            for w in range(nw):
                xt = p1.tile([128, 128], F32, tag="xt")
                nc.sync.dma_start(xt[:], xT[:, w * 128:(w + 1) * 128])
                xtb = p1.tile([128, 128], BF16, tag="xtb")
                nc.vector.tensor_copy(xtb[:], xt[:])
                ps = psA.tile([128, 400], F32, tag="p1ps")
                nc.tensor.matmul(ps[:], lhsT=xtb[:], rhs=wall[:],
                                 start=True, stop=True)
                for z, st in enumerate(sets):
                    o = z * 136
                    tbl = p1.tile([128, 128], F32, tag=f"tbl{z}")
                    tblb = tbl[:].bitcast(BF16)
                    nc.vector.tensor_copy(tblb[:, 0:128], ps[:, o:o + 128])
                    nc.vector.tensor_copy(tbl[:, 64:68], ps[:, o + 128:o + 132])
                    nc.vector.memset(tbl[:, 68:128], 0.0)
                    nc.vector.tensor_copy(tw[:, w, z * 4:z * 4 + 4],
                                          ps[:, o + 132:o + 136])
                    nc.sync.dma_start(st["ag_in"][w * 128:(w + 1) * 128, :], tbl[:])
                nc.scalar.activation(out_acc[:, w, :], ps[:, 272:400],
                                     ACTF.Copy, scale=eps_skip)

            for st in sets:
                if COLL_BYPASS:
                    nc.sync.dma_start(st["ag_out"][0:sh, :], st["ag_in"][:])
                else:
                    nc.gpsimd.collective_compute(
                        "AllGather", AL.bypass, replica_groups=rg,
                        ins=[st["ag_in"][:].opt()], outs=[st["ag_out"][:].opt()])

            # ---------------- edge phase ----------------
            for z, st in enumerate(sets):
                groups, calls = st["groups"], st["calls"]
                trel_f, idx_sb, ag_out = st["trel_f"], st["idx_sb"], st["ag_out"]

                # tile index -> (gather ring tile, position-in-call)
                tile_loc = {}
                call_of = {}
                for ci, (cc, t0, nt) in enumerate(calls):
                    g = gatp.tile([128, CALL_TILES * 128], F32, tag="gring")
                    dst = g[:, 0:nt * 128].rearrange("p (t e) -> p t e", e=128)
                    nidx = nt * 128
                    nc.gpsimd.dma_gather(
                        dst, ag_out[cc * chunk_rows:(cc + 1) * chunk_rows, :],
                        idx_sb[:, t0 * 8:t0 * 8 + nt * 8], nidx, nidx, 128,
                        queue_num=ci % 4)
                    for j in range(nt):
                        tile_loc[t0 + j] = (g, j)
                        call_of[t0 + j] = ci

                # per-call batched alpha/e_att/scale state
                call_state = {}

                def process_call(ci):
                    """S-compares, t-broadcasts, t-expand, alpha, exp, scale
                    for every tile of call ci — batched per call."""
                    cc, t0, nt = calls[ci]
                    g = tile_loc[t0][0]
                    Sc = workp.tile([128, CALL_TILES * 128], BF16, tag="S",
                                    name=f"S_{z}_{ci}", bufs=4)
                    iota3 = iota_bf[:].rearrange("p (o e) -> p o e", o=1)
                    trel3 = trel_f[:, t0:t0 + nt].rearrange(
                        "p (t o) -> p t o", o=1)
                    i3, t3 = bass.broadcast_tensor_aps(iota3, trel3)
                    nc.vector.tensor_tensor(
                        Sc[:, 0:nt * 128].rearrange("p (t e) -> p t e", e=128),
                        i3, t3, AL.is_equal)
                    # t-broadcast rows for the whole call (<=512 cols per mm)
                    stg = smallp.tile([1, CALL_TILES * 128], BF16, tag="trowstg",
                                      name=f"stg_{z}_{ci}")
                    nc.sync.dma_start(
                        stg[:, 0:nt * 128],
                        st["trow"][0:1, t0 * 128:(t0 + nt) * 128])
                    STc = workp.tile([128, CALL_TILES * 128], BF16, tag="STc",
                                     name=f"STc_{z}_{ci}", bufs=4)
                    for hi, mm0 in enumerate(range(0, nt * 128, 512)):
                        mm1 = min(mm0 + 512, nt * 128)
                        pbc = psC.tile([128, 512], F32, tag="pbc",
                                       name=f"pbc_{z}_{ci}_{hi}")
                        nc.tensor.matmul(pbc[:, 0:mm1 - mm0], lhsT=ones_row[:],
                                         rhs=stg[0:1, mm0:mm1],
                                         start=True, stop=True)
                        nc.vector.tensor_scalar(STc[:, mm0:mm1],
                                                pbc[:, 0:mm1 - mm0],
                                                iota_col[:], None, AL.is_equal)
                    # t-expand per tile (PE), results into one per-call bank
                    pte = psB.tile([128, CALL_TILES * 4], F32, tag="pte",
                                   name=f"pte_{z}_{ci}")
                    for j in range(nt):
                        wg = wg_of[t0 + j]
                        nc.tensor.matmul(
                            pte[:, j * 4:j * 4 + 4],
                            lhsT=STc[:, j * 128:(j + 1) * 128],
                            rhs=tw[:, wg, z * 4:z * 4 + 4],
                            start=True, stop=True)
                    # alpha = s + t ; lrelu ; exp -> B ; scale
                    al = smallp.tile([128, CALL_TILES * 4], F32, tag="al",
                                     name=f"al_{z}_{ci}")
                    alv = al[:, 0:nt * 4]
                    s_ap = g[:, 64:68]
                    s_ap3 = bass.AP(s_ap.tensor, s_ap.offset,
                                    [s_ap.ap[0], [128, nt], [1, 4]])
                    nc.vector.tensor_tensor(
                        alv.rearrange("p (t f) -> p t f", f=4), s_ap3,
                        pte[:, 0:nt * 4].rearrange("p (t f) -> p t f", f=4),
                        AL.add)
                    nc.vector.scalar_tensor_tensor(alv, alv, 0.01, alv,
                                                   AL.mult, AL.max)
                    B = workp.tile([128, CALL_TILES, 132], BF16, tag="B",
                                   name=f"B_{z}_{ci}", bufs=4)
                    nc.scalar.activation(
                        B[:, 0:nt, 128:132],
                        alv.rearrange("p (t f) -> p t f", f=4), ACTF.Exp)
                    gbf = g[:].bitcast(BF16)
                    mb = bass.AP(gbf.tensor, gbf.offset,
                                 [gbf.ap[0], [256, nt], [32, 4], [1, 32]])
                    b_sl = B[:, 0:nt, 128:132]
                    eb = bass.AP(b_sl.tensor, b_sl.offset,
                                 [*b_sl.ap, [0, 32]])
                    nc.vector.tensor_tensor(
                        B[:, 0:nt, 0:128].rearrange(
                            "p t (h c) -> p t h c", h=4), mb, eb, AL.mult)
                    call_state[ci] = (Sc, B)

                # scatter matmuls in stream order, windows accumulate in PSUM
                flushed = set()
                wg_of = {}
                for (cc, wg, cap, t0) in groups:
                    for j in range(cap):
                        wg_of[t0 + j] = wg
                for (cc, wg, cap, t0) in groups:
                    pw = psW.tile([128, 132], F32, tag="pw",
                                  name=f"pw_{z}_{cc}_{wg}")
                    for j in range(cap):
                        ci = call_of[t0 + j]
                        if ci not in call_state:
                            process_call(ci)
                            # retire old call states (ring depth)
                            for old in [k for k in call_state
                                        if k < ci - RING + 1]:
                                del call_state[old]
                        Sc, B = call_state[ci]
                        _, jj = tile_loc[t0 + j]
                        nc.tensor.matmul(pw[:],
                                         lhsT=Sc[:, jj * 128:(jj + 1) * 128],
                                         rhs=B[:, jj, :],
                                         start=(j == 0), stop=(j == cap - 1))
                    if wg not in flushed:
                        nc.scalar.copy(wacc[:, wg, :], pw[:])
                        flushed.add(wg)
                    else:
                        nc.vector.tensor_add(wacc[:, wg, :], wacc[:, wg, :],
                                             pw[:])

                # --- per-window epilogue: normalize + accumulate into out ---
                for wg in sorted(flushed):
                    den = smallp.tile([128, 4], F32, tag="den")
                    nc.vector.tensor_single_scalar(den[:], wacc[:, wg, 128:132],
                                                   1e-16, AL.add)
                    rec = smallp.tile([128, 4], F32, tag="rec")
                    nc.vector.reciprocal(rec[:], den[:])
                    tmp = smallp.tile([128, 128], F32, tag="tmp")
                    num = wacc[:, wg, 0:128].rearrange("p (h c) -> p h c", h=4)
                    recb = rec[:].rearrange("p (h o) -> p h o", o=1)
                    numb, recbb = bass.broadcast_tensor_aps(num, recb)
                    nc.vector.tensor_tensor(
                        tmp[:].rearrange("p (h c) -> p h c", h=4),
                        numb, recbb, AL.mult)
                    nc.vector.tensor_add(out_acc[:, wg, :], out_acc[:, wg, :],
                                         tmp[:])

            # ---------------- final relu + output ----------------
            oflat = out_acc[:].rearrange("p w c -> p (w c)")
            nc.vector.scalar_tensor_tensor(oflat, oflat, 0.0, oflat,
                                           AL.mult, AL.max)
            out_view = out_ext[:].rearrange("(w p) c -> p w c", p=128)
            nc.sync.dma_start(out_view, out_acc[:])

    nc.compile()
    return nc


# ----------------------------------------------------------------------------
# entry point
# ----------------------------------------------------------------------------

def _prepare(x, W_low, a_src_low, a_dst_low, W_up, a_src_up, a_dst_up, W_skip,
             lower_tgt, lower_src, upper_tgt, upper_src):
    n, inch = x.shape
    local = n // NCORES
    nw = (local + WIN - 1) // WIN
    sh = nw * WIN
    assert 3 * sh + local <= 32767, "int16 gather index overflow"

    lower_tgt = np.asarray(lower_tgt); lower_src = np.asarray(lower_src)
    upper_tgt = np.asarray(upper_tgt); upper_src = np.asarray(upper_src)
    perms = None
    if BINPACK:
        perms = []
        for k in range(NCORES):
            degs = []
            for tg, sr in ((lower_tgt, lower_src), (upper_tgt, upper_src)):
                m = (tg // local) == k
                tl = (tg[m] - k * local).astype(np.int64)
                cc = (sr[m].astype(np.int64) // local) // 4
                for c in range(2):
                    degs.append(np.bincount(tl[cc == c], minlength=local))
            perms.append(_binpack_windows(np.stack(degs), local, nw))
    capsL, groupsL, callsL, TL, coresL = _preprocess(
        lower_tgt, lower_src, local, nw, sh, perms)
    capsU, groupsU, callsU, TU, coresU = _preprocess(
        upper_tgt, upper_src, local, nw, sh, perms)

    meta = dict(nw=nw, sh=sh, eps_skip=1.0 + 1e-6,
                caps=[capsL, capsU], groups=[groupsL, groupsU],
                calls=[callsL, callsU], T=[TL, TU], perms=perms)

    wcat = np.concatenate([W_low, W_up, W_skip], axis=1).astype(np.float32)
    acat = np.concatenate(
        [_block_diag_a(np.asarray(a_src_low)), _block_diag_a(np.asarray(a_dst_low)),
         _block_diag_a(np.asarray(a_src_up)), _block_diag_a(np.asarray(a_dst_up))],
        axis=1).astype(np.float32)

    x = np.asarray(x, np.float32)
    in_maps = []
    for k in range(NCORES):
        xk = np.zeros((sh, inch), np.float32)
        if perms is not None:
            xk[perms[k]] = x[k * local:(k + 1) * local]
        else:
            xk[:local] = x[k * local:(k + 1) * local]
        in_maps.append({
            "xT": np.ascontiguousarray(xk.T),
            "Wcat": wcat, "Acat": acat,
            "idxL": coresL[k]["idx"], "trelL": coresL[k]["trel"],
            "trowL": coresL[k]["trow"],
            "idxU": coresU[k]["idx"], "trelU": coresU[k]["trel"],
            "trowU": coresU[k]["trow"],
        })
    return meta, in_maps, local, sh


def kernel(x, W_low, a_src_low, a_dst_low, W_up, a_src_up, a_dst_up, W_skip,
           lower_tgt, lower_src, upper_tgt, upper_src):
    from concourse.bass_utils import run_bass_kernel_spmd

    meta, in_maps, local, sh = _prepare(
        x, W_low, a_src_low, a_dst_low, W_up, a_src_up, a_dst_up, W_skip,
        lower_tgt, lower_src, upper_tgt, upper_src)
    nc = _build(meta)

    res = run_bass_kernel_spmd(nc, in_maps, list(range(NCORES)), trace=TRACE)
    LAST_RESULT["exec_time_ns"] = res.exec_time_ns
    LAST_RESULT["res"] = res

    n = np.asarray(x).shape[0]
    perms = meta["perms"]
    out = np.empty((n, 128), np.float32)
    for k in range(NCORES):
        ok = np.asarray(res.results[k]["out"])
        if perms is not None:
            out[k * local:(k + 1) * local] = ok[perms[k]]
        else:
            out[k * local:(k + 1) * local] = ok[:local]
    return out



# revision 6
# speedup vs baseline: 1.1717x; 1.1717x over previous
"""CANLayer (cell attention) distributed Bass kernel for 8 TRN2 NeuronCores.

Strategy (graph/data parallel by destination cell, per sharding hint):
 - core k owns target nodes [k*LOCAL, (k+1)*LOCAL)
 - phase 1: per-core fused matmul x_k @ [W | W@A_s | W@A_d] for both edge sets
   + skip; builds 512B gather-table rows [xm_bf16(256B) | s_f32(16B) | pad]
 - 2x AllGather replicates the per-set tables to every core's HBM
 - edge phase per set: edges sorted by (src-chunk, target-window); big
   dma_gather calls (one DMA descriptor per edge, full HBM BW); per 128-edge
   tile build one-hot S [e,t] via DVE compare vs iota; S^T via PE transpose
   expands per-target t-values to edges; alpha -> lrelu -> exp on batched
   groups; scatter = matmul(lhsT=S, rhs=[M*e_att | e_att]) accumulated in
   PSUM per 128-target window; normalize by the denominator columns.
"""
import sys

if "/opt/trn_rl_repo" not in sys.path:
    sys.path.insert(0, "/opt/trn_rl_repo")

import numpy as np

TRACE = False          # test.py sets True to get exec_time_ns + perfetto
LAST_RESULT = {}       # test.py reads exec_time_ns etc. from here

NCORES = 8
WIN = 128              # targets per PSUM window
CALL_TILES = 8         # max 128-edge tiles per dma_gather call
RING = 4               # gather ring depth (pool bufs)
BINPACK = True         # permute targets into windows to balance edge counts
T_ZERO_DEBUG = False   # diagnostic (legacy flag, unused by route B)
COLL_BYPASS = False    # diagnostic: replace AllGather with local shard copy
GATHER_SLIM = True     # gather only the 272B payload of each 512B table row


def _dma_gather_slim(gp, out_ap, in_ap, idxs_ap, num_idxs, num_idxs_reg,
                     elem_size, elem_step, queue_num=0):
    """nc.gpsimd.dma_gather with the elem_size%256B assert relaxed.

    The SWDGE ISA encodes the row stride in 256B units (elem_step must be a
    256B multiple) but the per-descriptor read size is a plain byte count;
    reading a 272B payload out of 512B-strided rows is legal at the
    descriptor level. Only HBM-source, transpose=False is supported here.
    """
    import concourse.mybir as mybir
    from concourse import ap_utils
    from concourse.bass_primitives import MemorySpace
    from concourse._compat import round_up_to_multiple, exact_div

    gp._assert_queue_num(queue_num)
    assert idxs_ap.dtype == mybir.dt.int16
    assert in_ap.dtype == out_ap.dtype
    assert in_ap.space == MemorySpace.DRAM
    assert idxs_ap.space == MemorySpace.SBUF
    assert out_ap.space == MemorySpace.SBUF
    assert ap_utils.ap_is_contiguous(out_ap.ap[1:])
    assert ap_utils.ap_is_contiguous(idxs_ap.ap[1:])
    assert in_ap.ap[-1][1] == out_ap.ap[-1][1] == elem_size
    assert out_ap.ap[0][1] * out_ap.ap[1][1] == round_up_to_multiple(num_idxs, 128)
    assert in_ap.ap[0][0] == elem_step
    stride_bytes = elem_step * mybir.dt.size(in_ap.dtype)
    stride_bytes_256 = exact_div(stride_bytes, 256)
    assert stride_bytes_256 < 256

    _in_ap = gp.lower_ap_dma(in_ap, for_custom_bir_dma=True)
    _idxs_ap = gp.lower_ap(idxs_ap)
    _out_ap = gp.lower_ap(out_ap)
    return gp.add_instruction(
        mybir.InstDMAGatherAnt(
            name=gp.bass.get_next_instruction_name(),
            ins=[*_in_ap, _idxs_ap,
                 gp.lower_val_access(gp.to_reg(num_idxs_reg))],
            outs=[_out_ap],
            transpose=False,
            num_idxs=num_idxs,
            elem_size=elem_size,
            stride_bytes_256=stride_bytes_256,
            gen_mode=0,
            single_packet=True,
            queue_num=queue_num,
            sbuf_tokens_per_rank=0,
            sbuf_free_dim_per_rank=0,
            sbuf_free_dim_pad_per_rank=0,
            sbuf_byte_offset=0,
        )
    )


def _binpack_windows(degs, local, nw):
    """degs: [n_kinds, local] per-target edge counts. Returns perm: array
    [local] -> window-major new position, such that each window's per-kind
    counts stay near/below budget. Greedy first-fit-decreasing."""
    nk = degs.shape[0]
    budgets = np.full((nw, nk), 500, np.int64)
    budgets[max(0, nw - 8):] = 6 * WIN
    counts = np.zeros((nw, nk), np.int64)
    nslots = np.zeros(nw, np.int64)
    win_of = np.full(local, -1, np.int64)
    order = np.argsort(-degs.sum(0), kind="stable")
    cap_slots = np.full(nw, WIN, np.int64)
    cap_slots[nw - 1] = local - (nw - 1) * WIN
    for t in order:
        d = degs[:, t]
        best, best_cost = -1, None
        for w in range(nw):
            if nslots[w] >= cap_slots[w]:
                continue
            over = np.maximum(counts[w] + d - budgets[w], 0).sum()
            slack = (budgets[w] - counts[w] - d).sum()
            cost = (over * 100000) - slack
            if best_cost is None or cost < best_cost:
                best, best_cost = w, cost
        counts[best] += d
        win_of[t] = best
        nslots[best] += 1
    # position within window
    perm = np.empty(local, np.int64)
    used = np.zeros(nw, np.int64)
    for t in range(local):
        w = win_of[t]
        perm[t] = w * WIN + used[w]
        used[w] += 1
    return perm


# ----------------------------------------------------------------------------
# host-side index preprocessing (pure layout/index manipulation)
# ----------------------------------------------------------------------------

def _preprocess(tgt, src, local, nw, sh, perms=None):
    """Shard one edge set by destination core; sort by (src-chunk, window).

    Static structure (caps, groups, calls) is shared across cores (maxed)
    as required for SPMD; per-core arrays carry indices + relative targets.
    perms[k] (optional) maps local target id -> permuted slot id.
    """
    per_core = []
    for k in range(NCORES):
        m = (tgt // local) == k
        s = src[m].astype(np.int64)
        tl = (tgt[m] - k * local).astype(np.int64)
        if perms is not None:
            tl = perms[k][tl]
        w = tl // WIN
        c = (s // local) // 4
        order = np.lexsort((tl, w, c))
        per_core.append((s[order], tl[order], w[order], c[order]))

    caps = np.zeros((2, nw), np.int64)
    for k in range(NCORES):
        s, tl, w, c = per_core[k]
        for cc in range(2):
            cnt = np.bincount(w[c == cc], minlength=nw)
            caps[cc] = np.maximum(caps[cc], (cnt + WIN - 1) // WIN)

    # groups in stream order; calls are CALL_TILES-sized slices of each
    # chunk run (groups may span calls)
    groups = []          # (cc, w, cap, tile_start)
    calls = []           # (cc, tile_start, n_tiles)
    t_idx = 0
    for cc in range(2):
        run_start = t_idx
        for w in range(nw):
            cap = int(caps[cc][w])
            if cap == 0:
                continue
            groups.append((cc, w, cap, t_idx))
            t_idx += cap
        for c0 in range(run_start, t_idx, CALL_TILES):
            calls.append((cc, c0, min(CALL_TILES, t_idx - c0)))
    T = t_idx

    cores = []
    for k in range(NCORES):
        s, tl, w, c = per_core[k]
        slots = T * 128
        src16 = np.zeros(slots, np.int16)          # pad -> idx 0 (valid row)
        trel = np.full(slots, -1.0, np.float32)    # pad -> -1 (no S match)
        for (cc, wg, cap, t0) in groups:
            sel = (c == cc) & (w == wg)
            n = int(sel.sum())
            off = t0 * 128
            ss = s[sel]
            if perms is not None:
                sloc = np.empty(len(ss), np.int64)
                for ks in range(NCORES):
                    mm = (ss // local) == ks
                    sloc[mm] = perms[ks][ss[mm] % local]
            else:
                sloc = ss % local
            src16[off:off + n] = (((ss // local) % 4) * sh + sloc).astype(np.int16)
            trel[off:off + n] = (tl[sel] - wg * WIN).astype(np.float32)
        tloc = np.zeros(slots, np.int32)
        for (cc, wg, cap, t0) in groups:
            blk = slice(t0 * 128, (t0 + cap) * 128)
            tloc[blk] = np.maximum(trel[blk].astype(np.int32), 0) + wg * WIN
        ii = np.arange(slots)
        idxarr = np.zeros((128, T * 8), np.int16)
        for g8 in range(8):
            idxarr[g8 * 16 + ii % 16, ii // 16] = src16
        trelarr = np.full((128, T), -1.0, np.float32)
        trelarr[ii % 128, ii // 128] = trel
        import ml_dtypes
        cores.append({"idx": idxarr, "trel": trelarr,
                      "trow": trel.reshape(1, T * 128).astype(ml_dtypes.bfloat16)})
    return caps, groups, calls, T, cores


def _block_diag_a(a):  # [H, C] -> [H*C, H] block diagonal (layout only)
    h, c = a.shape
    out = np.zeros((h * c, h), np.float32)
    for i in range(h):
        out[i * c:(i + 1) * c, i] = a[i]
    return out


# ----------------------------------------------------------------------------
# device kernel builder
# ----------------------------------------------------------------------------

def _build(meta):
    import concourse.bass as bass
    import concourse.bacc as bacc
    import concourse.mybir as mybir
    import concourse.tile as tile

    F32 = mybir.dt.float32
    BF16 = mybir.dt.bfloat16
    I16 = mybir.dt.int16
    I32 = mybir.dt.int32
    AL = mybir.AluOpType
    ACTF = mybir.ActivationFunctionType

    nw, sh = meta["nw"], meta["sh"]
    chunk_rows = 4 * sh
    eps_skip = meta["eps_skip"]

    nc = bacc.Bacc("TRN2", target_bir_lowering=False, debug=False,
                   num_devices=NCORES, num_swdge_queues=4)

    xT = nc.dram_tensor("xT", [128, sh], F32, kind="ExternalInput")
    Wcat = nc.dram_tensor("Wcat", [128, 384], F32, kind="ExternalInput")
    Acat = nc.dram_tensor("Acat", [128, 16], F32, kind="ExternalInput")
    out_ext = nc.dram_tensor("out", [sh, 128], F32, kind="ExternalOutput")

    sets = []
    for z, zn in enumerate("LU"):
        TZ = meta["T"][z]
        sets.append(dict(
            z=z, zn=zn,
            idx=nc.dram_tensor(f"idx{zn}", [128, TZ * 8], I16, kind="ExternalInput"),
            trel=nc.dram_tensor(f"trel{zn}", [128, TZ], F32, kind="ExternalInput"),
            trow=nc.dram_tensor(f"trow{zn}", [1, TZ * 128], BF16, kind="ExternalInput"),
            ag_in=nc.dram_tensor(f"agin{zn}", [sh, 128], F32),
            ag_out=nc.dram_tensor(f"agout{zn}", [NCORES * sh, 128], F32,
                                  addr_space="Shared"),
            caps=meta["caps"][z], groups=meta["groups"][z],
            calls=meta["calls"][z], T=TZ,
        ))

    rg = [list(range(NCORES))]

    with tile.TileContext(nc) as tc:
        with (
            tc.tile_pool(name="const", bufs=1) as constp,
            tc.tile_pool(name="p1", bufs=3) as p1,
            tc.tile_pool(name="gat", bufs=RING) as gatp,
            tc.tile_pool(name="work", bufs=6) as workp,
            tc.tile_pool(name="small", bufs=4) as smallp,
            tc.tile_pool(name="winb", bufs=1) as winp,
            tc.tile_pool(name="psA", bufs=2, space="PSUM") as psA,
            tc.tile_pool(name="psB", bufs=2, space="PSUM") as psB,
            tc.tile_pool(name="psC", bufs=2, space="PSUM") as psC,
            tc.tile_pool(name="psW", bufs=2, space="PSUM") as psW,
        ):
            # ---------------- constants ----------------
            wcat = constp.tile([128, 384], F32)
            nc.sync.dma_start(wcat[:], Wcat[:])
            wcat_bf = constp.tile([128, 384], BF16)
            nc.vector.tensor_copy(wcat_bf[:], wcat[:])
            acat = constp.tile([128, 16], F32)
            nc.sync.dma_start(acat[:], Acat[:])
            acat_bf = constp.tile([128, 16], BF16)
            nc.vector.tensor_copy(acat_bf[:], acat[:])

            iota_i = constp.tile([128, 128], I32)
            nc.gpsimd.iota(iota_i[:], [[1, 128]], base=0, channel_multiplier=0)
            iota_bf = constp.tile([128, 128], BF16)
            nc.vector.tensor_copy(iota_bf[:], iota_i[:])
            ones_row = constp.tile([1, 128], BF16)
            nc.vector.memset(ones_row[:], 1.0)
            iota_col = constp.tile([128, 1], F32)
            nc.gpsimd.iota(iota_col[:].bitcast(I32), [[1, 1]], base=0,
                           channel_multiplier=1)
            nc.vector.tensor_copy(iota_col[:], iota_col[:].bitcast(I32))
            iodiag = constp.tile([128, 128], I32)
            nc.gpsimd.iota(iodiag[:], [[1, 128]], base=0, channel_multiplier=-1)
            ident_bf = constp.tile([128, 128], BF16)
            nc.vector.tensor_single_scalar(ident_bf[:], iodiag[:], 0.0, AL.is_equal)

            for st in sets:
                zn = st["zn"]
                st["idx_sb"] = constp.tile([128, st["T"] * 8], I16, tag=f"idxsb{zn}", name=f"idxsb{zn}")
                nc.sync.dma_start(st["idx_sb"][:], st["idx"][:])
                st["trel_f"] = constp.tile([128, st["T"]], F32, tag=f"trelf{zn}", name=f"trelf{zn}")
                nc.sync.dma_start(st["trel_f"][:], st["trel"][:])


            # W_all = [Wl | Wl@As_l | Wl@Ad_l | Wu | ... | Wskip] bf16 [128,400]
            wall = constp.tile([128, 400], BF16)
            nc.vector.tensor_copy(wall[:, 0:128], wcat[:, 0:128])
            nc.vector.tensor_copy(wall[:, 136:264], wcat[:, 128:256])
            nc.vector.tensor_copy(wall[:, 272:400], wcat[:, 256:384])
            for z in range(2):
                pst = psC.tile([128, 128], BF16, tag="pbc")
                nc.tensor.transpose(pst[:], wcat_bf[:, z * 128:(z + 1) * 128],
                                    ident_bf[:])
                wTb = smallp.tile([128, 128], BF16, tag="wTb")
                nc.scalar.copy(wTb[:], pst[:])
                pwa = psW.tile([128, 132], F32, tag="pw")
                nc.tensor.matmul(pwa[:, 0:8], lhsT=wTb[:],
                                 rhs=acat_bf[:, z * 8:(z + 1) * 8],
                                 start=True, stop=True)
                nc.vector.tensor_copy(wall[:, 128 + z * 136:136 + z * 136],
                                      pwa[:, 0:8])

            # ---------------- persistent buffers ----------------
            out_acc = winp.tile([128, nw, 128], F32)
            tw = winp.tile([128, nw, 8], BF16)
            wacc = winp.tile([128, nw, 132], F32)

            # ---------------- phase 1 ----------------
            for w in range(nw):
                xt = p1.tile([128, 128], F32, tag="xt")
                nc.sync.dma_start(xt[:], xT[:, w * 128:(w + 1) * 128])
                xtb = p1.tile([128, 128], BF16, tag="xtb")
                nc.vector.tensor_copy(xtb[:], xt[:])
                ps = psA.tile([128, 400], F32, tag="p1ps")
                nc.tensor.matmul(ps[:], lhsT=xtb[:], rhs=wall[:],
                                 start=True, stop=True)
                for z, st in enumerate(sets):
                    o = z * 136
                    tbl = p1.tile([128, 128], F32, tag=f"tbl{z}")
                    tblb = tbl[:].bitcast(BF16)
                    nc.vector.tensor_copy(tblb[:, 0:128], ps[:, o:o + 128])
                    nc.vector.tensor_copy(tbl[:, 64:68], ps[:, o + 128:o + 132])
                    nc.vector.memset(tbl[:, 68:128], 0.0)
                    nc.vector.tensor_copy(tw[:, w, z * 4:z * 4 + 4],
                                          ps[:, o + 132:o + 136])
                    nc.sync.dma_start(st["ag_in"][w * 128:(w + 1) * 128, :], tbl[:])
                nc.scalar.activation(out_acc[:, w, :], ps[:, 272:400],
                                     ACTF.Copy, scale=eps_skip)

            for st in sets:
                if COLL_BYPASS:
                    nc.sync.dma_start(st["ag_out"][0:sh, :], st["ag_in"][:])
                else:
                    nc.gpsimd.collective_compute(
                        "AllGather", AL.bypass, replica_groups=rg,
                        ins=[st["ag_in"][:].opt()], outs=[st["ag_out"][:].opt()])

            # ---------------- edge phase ----------------
            for z, st in enumerate(sets):
                groups, calls = st["groups"], st["calls"]
                trel_f, idx_sb, ag_out = st["trel_f"], st["idx_sb"], st["ag_out"]

                # tile index -> (gather ring tile, position-in-call)
                EW = 68 if GATHER_SLIM else 128   # f32 cols gathered per edge
                tile_loc = {}
                call_of = {}
                for ci, (cc, t0, nt) in enumerate(calls):
                    g = gatp.tile([128, CALL_TILES * EW], F32, tag="gring")
                    dst = g[:, 0:nt * EW].rearrange("p (t e) -> p t e", e=EW)
                    nidx = nt * 128
                    if GATHER_SLIM:
                        _dma_gather_slim(
                            nc.gpsimd, dst,
                            ag_out[cc * chunk_rows:(cc + 1) * chunk_rows, 0:EW],
                            idx_sb[:, t0 * 8:t0 * 8 + nt * 8], nidx, nidx, EW,
                            128, queue_num=ci % 4)
                    else:
                        nc.gpsimd.dma_gather(
                            dst, ag_out[cc * chunk_rows:(cc + 1) * chunk_rows, :],
                            idx_sb[:, t0 * 8:t0 * 8 + nt * 8], nidx, nidx, 128,
                            queue_num=ci % 4)
                    for j in range(nt):
                        tile_loc[t0 + j] = (g, j)
                        call_of[t0 + j] = ci

                # per-call batched alpha/e_att/scale state
                call_state = {}

                def process_call(ci):
                    """S-compares, t-broadcasts, t-expand, alpha, exp, scale
                    for every tile of call ci — batched per call."""
                    cc, t0, nt = calls[ci]
                    g = tile_loc[t0][0]
                    Sc = workp.tile([128, CALL_TILES * 128], BF16, tag="S",
                                    name=f"S_{z}_{ci}", bufs=4)
                    iota3 = iota_bf[:].rearrange("p (o e) -> p o e", o=1)
                    trel3 = trel_f[:, t0:t0 + nt].rearrange(
                        "p (t o) -> p t o", o=1)
                    i3, t3 = bass.broadcast_tensor_aps(iota3, trel3)
                    nc.vector.tensor_tensor(
                        Sc[:, 0:nt * 128].rearrange("p (t e) -> p t e", e=128),
                        i3, t3, AL.is_equal)
                    # t-broadcast rows for the whole call (<=512 cols per mm)
                    stg = smallp.tile([1, CALL_TILES * 128], BF16, tag="trowstg",
                                      name=f"stg_{z}_{ci}")
                    nc.sync.dma_start(
                        stg[:, 0:nt * 128],
                        st["trow"][0:1, t0 * 128:(t0 + nt) * 128])
                    STc = workp.tile([128, CALL_TILES * 128], BF16, tag="STc",
                                     name=f"STc_{z}_{ci}", bufs=4)
                    for hi, mm0 in enumerate(range(0, nt * 128, 512)):
                        mm1 = min(mm0 + 512, nt * 128)
                        pbc = psC.tile([128, 512], F32, tag="pbc",
                                       name=f"pbc_{z}_{ci}_{hi}")
                        nc.tensor.matmul(pbc[:, 0:mm1 - mm0], lhsT=ones_row[:],
                                         rhs=stg[0:1, mm0:mm1],
                                         start=True, stop=True)
                        nc.vector.tensor_scalar(STc[:, mm0:mm1],
                                                pbc[:, 0:mm1 - mm0],
                                                iota_col[:], None, AL.is_equal)
                    # t-expand per tile (PE), results into one per-call bank
                    pte = psB.tile([128, CALL_TILES * 4], F32, tag="pte",
                                   name=f"pte_{z}_{ci}")
                    for j in range(nt):
                        wg = wg_of[t0 + j]
                        nc.tensor.matmul(
                            pte[:, j * 4:j * 4 + 4],
                            lhsT=STc[:, j * 128:(j + 1) * 128],
                            rhs=tw[:, wg, z * 4:z * 4 + 4],
                            start=True, stop=True)
                    # alpha = s + t ; lrelu ; exp -> B ; scale
                    al = smallp.tile([128, CALL_TILES * 4], F32, tag="al",
                                     name=f"al_{z}_{ci}")
                    alv = al[:, 0:nt * 4]
                    s_ap = g[:, 64:68]
                    s_ap3 = bass.AP(s_ap.tensor, s_ap.offset,
                                    [s_ap.ap[0], [EW, nt], [1, 4]])
                    nc.vector.tensor_tensor(
                        alv.rearrange("p (t f) -> p t f", f=4), s_ap3,
                        pte[:, 0:nt * 4].rearrange("p (t f) -> p t f", f=4),
                        AL.add)
                    nc.vector.scalar_tensor_tensor(alv, alv, 0.01, alv,
                                                   AL.mult, AL.max)
                    B = workp.tile([128, CALL_TILES, 132], BF16, tag="B",
                                   name=f"B_{z}_{ci}", bufs=4)
                    nc.scalar.activation(
                        B[:, 0:nt, 128:132],
                        alv.rearrange("p (t f) -> p t f", f=4), ACTF.Exp)
                    gbf = g[:].bitcast(BF16)
                    mb = bass.AP(gbf.tensor, gbf.offset,
                                 [gbf.ap[0], [2 * EW, nt], [32, 4], [1, 32]])
                    b_sl = B[:, 0:nt, 128:132]
                    eb = bass.AP(b_sl.tensor, b_sl.offset,
                                 [*b_sl.ap, [0, 32]])
                    nc.vector.tensor_tensor(
                        B[:, 0:nt, 0:128].rearrange(
                            "p t (h c) -> p t h c", h=4), mb, eb, AL.mult)
                    call_state[ci] = (Sc, B)

                # scatter matmuls in stream order, windows accumulate in PSUM
                flushed = set()
                wg_of = {}
                for (cc, wg, cap, t0) in groups:
                    for j in range(cap):
                        wg_of[t0 + j] = wg
                for (cc, wg, cap, t0) in groups:
                    pw = psW.tile([128, 132], F32, tag="pw",
                                  name=f"pw_{z}_{cc}_{wg}")
                    for j in range(cap):
                        ci = call_of[t0 + j]
                        if ci not in call_state:
                            process_call(ci)
                            # retire old call states (ring depth)
                            for old in [k for k in call_state
                                        if k < ci - RING + 1]:
                                del call_state[old]
                        Sc, B = call_state[ci]
                        _, jj = tile_loc[t0 + j]
                        nc.tensor.matmul(pw[:],
                                         lhsT=Sc[:, jj * 128:(jj + 1) * 128],
                                         rhs=B[:, jj, :],
                                         start=(j == 0), stop=(j == cap - 1))
                    if wg not in flushed:
                        nc.scalar.copy(wacc[:, wg, :], pw[:])
                        flushed.add(wg)
                    else:
                        nc.vector.tensor_add(wacc[:, wg, :], wacc[:, wg, :],
                                             pw[:])

                # --- per-window epilogue: normalize + accumulate into out ---
                for wg in sorted(flushed):
                    den = smallp.tile([128, 4], F32, tag="den")
                    nc.vector.tensor_single_scalar(den[:], wacc[:, wg, 128:132],
                                                   1e-16, AL.add)
                    rec = smallp.tile([128, 4], F32, tag="rec")
                    nc.vector.reciprocal(rec[:], den[:])
                    tmp = smallp.tile([128, 128], F32, tag="tmp")
                    num = wacc[:, wg, 0:128].rearrange("p (h c) -> p h c", h=4)
                    recb = rec[:].rearrange("p (h o) -> p h o", o=1)
                    numb, recbb = bass.broadcast_tensor_aps(num, recb)
                    nc.vector.tensor_tensor(
                        tmp[:].rearrange("p (h c) -> p h c", h=4),
                        numb, recbb, AL.mult)
                    nc.vector.tensor_add(out_acc[:, wg, :], out_acc[:, wg, :],
                                         tmp[:])

            # ---------------- final relu + output ----------------
            oflat = out_acc[:].rearrange("p w c -> p (w c)")
            nc.vector.scalar_tensor_tensor(oflat, oflat, 0.0, oflat,
                                           AL.mult, AL.max)
            out_view = out_ext[:].rearrange("(w p) c -> p w c", p=128)
            nc.sync.dma_start(out_view, out_acc[:])

    nc.compile()
    return nc


# ----------------------------------------------------------------------------
# entry point
# ----------------------------------------------------------------------------

def _prepare(x, W_low, a_src_low, a_dst_low, W_up, a_src_up, a_dst_up, W_skip,
             lower_tgt, lower_src, upper_tgt, upper_src):
    n, inch = x.shape
    local = n // NCORES
    nw = (local + WIN - 1) // WIN
    sh = nw * WIN
    assert 3 * sh + local <= 32767, "int16 gather index overflow"

    lower_tgt = np.asarray(lower_tgt); lower_src = np.asarray(lower_src)
    upper_tgt = np.asarray(upper_tgt); upper_src = np.asarray(upper_src)
    perms = None
    if BINPACK:
        perms = []
        for k in range(NCORES):
            degs = []
            for tg, sr in ((lower_tgt, lower_src), (upper_tgt, upper_src)):
                m = (tg // local) == k
                tl = (tg[m] - k * local).astype(np.int64)
                cc = (sr[m].astype(np.int64) // local) // 4
                for c in range(2):
                    degs.append(np.bincount(tl[cc == c], minlength=local))
            perms.append(_binpack_windows(np.stack(degs), local, nw))
    capsL, groupsL, callsL, TL, coresL = _preprocess(
        lower_tgt, lower_src, local, nw, sh, perms)
    capsU, groupsU, callsU, TU, coresU = _preprocess(
        upper_tgt, upper_src, local, nw, sh, perms)

    meta = dict(nw=nw, sh=sh, eps_skip=1.0 + 1e-6,
                caps=[capsL, capsU], groups=[groupsL, groupsU],
                calls=[callsL, callsU], T=[TL, TU], perms=perms)

    wcat = np.concatenate([W_low, W_up, W_skip], axis=1).astype(np.float32)
    acat = np.concatenate(
        [_block_diag_a(np.asarray(a_src_low)), _block_diag_a(np.asarray(a_dst_low)),
         _block_diag_a(np.asarray(a_src_up)), _block_diag_a(np.asarray(a_dst_up))],
        axis=1).astype(np.float32)

    x = np.asarray(x, np.float32)
    in_maps = []
    for k in range(NCORES):
        xk = np.zeros((sh, inch), np.float32)
        if perms is not None:
            xk[perms[k]] = x[k * local:(k + 1) * local]
        else:
            xk[:local] = x[k * local:(k + 1) * local]
        in_maps.append({
            "xT": np.ascontiguousarray(xk.T),
            "Wcat": wcat, "Acat": acat,
            "idxL": coresL[k]["idx"], "trelL": coresL[k]["trel"],
            "trowL": coresL[k]["trow"],
            "idxU": coresU[k]["idx"], "trelU": coresU[k]["trel"],
            "trowU": coresU[k]["trow"],
        })
    return meta, in_maps, local, sh


def kernel(x, W_low, a_src_low, a_dst_low, W_up, a_src_up, a_dst_up, W_skip,
           lower_tgt, lower_src, upper_tgt, upper_src):
    from concourse.bass_utils import run_bass_kernel_spmd

    meta, in_maps, local, sh = _prepare(
        x, W_low, a_src_low, a_dst_low, W_up, a_src_up, a_dst_up, W_skip,
        lower_tgt, lower_src, upper_tgt, upper_src)
    nc = _build(meta)

    res = run_bass_kernel_spmd(nc, in_maps, list(range(NCORES)), trace=TRACE)
    LAST_RESULT["exec_time_ns"] = res.exec_time_ns
    LAST_RESULT["res"] = res

    n = np.asarray(x).shape[0]
    perms = meta["perms"]
    out = np.empty((n, 128), np.float32)
    for k in range(NCORES):
        ok = np.asarray(res.results[k]["out"])
        if perms is not None:
            out[k * local:(k + 1) * local] = ok[perms[k]]
        else:
            out[k * local:(k + 1) * local] = ok[:local]
    return out

